# revision 8
# baseline (speedup 1.0000x reference)
"""BranchRoute (2-branch threshold MoE routing) Trainium2 kernel.

Full-input contract: kernel(x, gate_w, gate_b) -> (x0, x1, combined),
x: [8192, 4096] f32, gate_w: [4096, 2] f32, gate_b: [2] f32.

Math: z = x @ gate_w; m_i = z_i > -b_i  (== sigmoid(z_i + b_i) > 0.5);
x0 = x * m0, x1 = x * m1, combined = x * (m0 + m1).

Sharding: data-parallel over tokens, 8 shards of 1024 tokens, one per
NeuronCore; gate weights replicated; no cross-core communication.

Memory-bound problem, so the kernel minimizes HBM traffic and DMA
instruction count:

  * Outputs are stored as float16 (the correctness budget is generous:
    fp16 round-off is ~3e-4 norm-relative) and widened to f32 on the
    host during the unshard. Write traffic halves: 48 -> 24 MiB/core.
  * The three per-tile outputs live interleaved in one SBUF tensor
    [128, 3*4096] f16 and in one DRAM tensor [SHARD, 3*4096] f16, so
    each tile needs exactly ONE 3 MiB store (columns 0:D = x0,
    D:2D = x1, 2D:3D = combined; host splits via reshape).
  * Per tile: one 2 MiB x load (ACT HWDGE ring, issued by the scalar
    engine) + one 3 MiB store (SP HWDGE ring). 5 MiB/tile total,
    40 MiB/core -> ~106 us at the ~394 GB/s/core the baseline
    f32 kernel measured.

Engine split (per 128-token tile, all under the ~13 us DMA time):
  DVE: two scalar_tensor_tensor ops (prod = x*w into PSUM scratch with
    accum_out -> z, fusing the old mult+ACT-reduce pair), the is_gt
    mask, m0+m1, and the x0 output (tensor_scalar f32 2x mode).
  ACT: the x1 and combined outputs (Copy with per-partition scale,
    1 elem/cycle @ 1.2 GHz).

Raw Bass (no Tile: the local walrus build encodes at most ONE sem wait
per instruction). Per-slot DMA semaphores so every semaphore tracks at
most one outstanding transfer and waits are unambiguous.
"""

import sys

import numpy as np

sys.path.insert(0, "/opt/trn_rl_repo")

import concourse.bass as bass
from concourse import mybir
from concourse.bass_utils import run_bass_kernel_spmd

N_CORES = 8
N, D = 8192, 4096
SHARD = N // N_CORES  # 1024 tokens per core
P = 128
NT = SHARD // P  # 8 tiles per core
F32 = mybir.dt.float32
F16 = mybir.dt.float16
Copy = mybir.ActivationFunctionType.Copy
Alu = mybir.AluOpType

_CACHE = {}


def _build(nt=NT, n_pass=1):
    nc = bass.Bass()
    x_in = nc.dram_tensor("x", [SHARD, D], F32, kind="ExternalInput")
    gw_in = nc.dram_tensor("gate_w", [D, 2], F32, kind="ExternalInput")
    gb_in = nc.dram_tensor("gate_b", [2], F32, kind="ExternalInput")
    out_d = nc.dram_tensor("out", [SHARD, 3 * D], F16, kind="ExternalOutput")

    NPT = nt * n_pass  # total tile iterations (n_pass > 1: timing loops)

    def tid(it):  # tile row index within the shard for iteration it
        return it % nt

    from contextlib import ExitStack

    with ExitStack() as ctx:
        sb = lambda name, *shape, dt=F32: ctx.enter_context(
            nc.sbuf_tensor(name, list(shape), dt)
        )
        sem = lambda name: ctx.enter_context(nc.semaphore(name))
        gwb = sb("gwb", P, 2 * D)  # interleaved w0/w1 bcast
        bb = sb("bb", P, 2)  # bias bcast
        nb = sb("nb", P, 2)  # -bias
        xt = [sb(f"xt{i}", P, D) for i in range(3)]
        osl = [sb(f"osl{i}", P, 3 * D, dt=F16) for i in range(3)]
        prod0 = ctx.enter_context(nc.psum_tensor("prod0", [P, D], F32))
        prod1 = sb("prod1", P, D)
        zq = [sb(f"zq{q}", P, 2) for q in range(2)]  # z dbl-buf
        mk = sb("mk", P, 3)  # m0|m1|ms (DVE-local, single buffer)
        setup_sem = sem("setup_sem")
        inx = [sem(f"inx{i}") for i in range(3)]
        sout = [sem(f"sout{i}") for i in range(3)]
        vec_sem = sem("vec_sem")
        act_sem = sem("act_sem")
        block = ctx.enter_context(nc.Block())
        # de-interleaved strided views of the broadcast weights [P, D]
        gw_v = gwb[:].rearrange("p (d t) -> p t d", t=2)
        w0v = gw_v[:, 0:1, :].rearrange("p one d -> p (one d)")
        w1v = gw_v[:, 1:2, :].rearrange("p one d -> p (one d)")

        # Software-pipelined by one tile: DVE iteration `it` runs the two
        # gate multiplies for tile `it`, then masks+outputs for tile it-1
        # (whose z the ACT engine reduced meanwhile). ACT does only the
        # two in-place Copy reductions (accum_out -> z); all three masked
        # outputs are DVE tensor_scalar ops (f32->f16, 2x mode), so one
        # DVE drain fences the whole osl slot before the store.
        #
        # A fused mult+accum (scalar_tensor_tensor w/ accum_out) on DVE
        # measured ~2.3x slower than plain tensor_mul once the output
        # stage runs concurrently - hence the mult/ACT-reduce split.
        #
        # Semaphores: vec_sem/act_sem are op counters recorded in VC/AC
        # dicts at emission time; inx[s]/sout[s] are per-slot DMA sems.
        VC = {}  # (tag, it) -> vec_sem value when that op completes
        vc = [0]

        def v_inc(instr, tag, it):
            instr.then_inc(vec_sem, 1)
            vc[0] += 1
            VC[(tag, it)] = vc[0]

        AC = lambda it, k: 2 * it + k  # act_sem: 2 reductions/tile

        def x_done(it):  # x-load completions for slot it%3 up to tile it
            return 16 * (it // 3 + 1)

        def slot_free(it):  # store completions needed so slot it%3 is free
            return 16 * (it // 3)

        def n_stores(j):  # stores on slot j over the whole program
            return (NPT - j + 2) // 3

        @block.vector
        def _(vector):
            vector.wait_ge(setup_sem, 32)
            v_inc(
                nc.vector.tensor_scalar_mul(nb[:], bb[:], -1.0), "nb", 0
            )

            def masks_outputs(j):
                sj = j % 3
                qj = j % 2
                vector.wait_ge(act_sem, AC(j, 2))  # z[qj] ready
                v_inc(
                    nc.vector.tensor_tensor(
                        mk[:, 0:2], zq[qj][:, 0:2], nb[:, 0:2], Alu.is_gt
                    ),
                    "isgt", j,
                )
                vector.wait_ge(vec_sem, VC[("isgt", j)])  # m drained
                v_inc(
                    nc.vector.tensor_add(
                        mk[:, 2:3], mk[:, 0:1], mk[:, 1:2]
                    ),
                    "add", j,
                )
                vector.wait_ge(vec_sem, VC[("add", j)])  # ms drained
                if j >= 3:
                    vector.wait_ge(sout[sj], slot_free(j))  # osl[sj] stored
                v_inc(
                    nc.vector.tensor_scalar_mul(
                        osl[sj][:, 0:D], xt[sj][:], mk[:, 0:1]
                    ),
                    "o0", j,
                )
                v_inc(
                    nc.vector.tensor_scalar_mul(
                        osl[sj][:, D : 2 * D], xt[sj][:], mk[:, 1:2]
                    ),
                    "o1", j,
                )
                v_inc(
                    nc.vector.tensor_scalar_mul(
                        osl[sj][:, 2 * D : 3 * D], xt[sj][:], mk[:, 2:3]
                    ),
                    "oc", j,
                )
                # write fence: the store must not read osl[sj] until the
                # o* writes drained (an op's then_inc can fire while its
                # SBUF writes are still landing)
                v_inc(nc.vector.drain(), "fence", j)

            for it in range(NPT):
                s = it % 3
                vector.wait_ge(inx[s], x_done(it))
                if it >= 1:
                    vector.wait_ge(act_sem, AC(it - 1, 1))  # prod0 free
                v_inc(
                    nc.vector.tensor_mul(prod0[:], xt[s][:], w0v), "m0", it
                )
                if it >= 1:
                    vector.wait_ge(act_sem, AC(it - 1, 2))  # prod1 free
                v_inc(
                    nc.vector.tensor_mul(prod1[:], xt[s][:], w1v), "m1", it
                )
                if it >= 1:
                    masks_outputs(it - 1)
            masks_outputs(NPT - 1)

        @block.scalar
        def _(scalar):
            # x loads ride the Activation HWDGE ring so they never queue
            # behind store waits on the SP ring.
            for it in range(min(3, NPT)):
                r = bass.ts(tid(it), P)
                scalar.dma_start(xt[it][:], x_in[r, :]).then_inc(inx[it], 16)
            for it in range(NPT):
                s = it % 3
                q = it % 2
                # zq[q] overwrite: its previous reader is isgt(it-2)
                if it >= 2:
                    scalar.wait_ge(vec_sem, VC[("isgt", it - 2)])
                scalar.wait_ge(vec_sem, VC[("m0", it)])
                nc.scalar.activation(
                    prod0[:], prod0[:], Copy, accum_out=zq[q][:, 0:1]
                ).then_inc(act_sem, 1)
                scalar.wait_ge(vec_sem, VC[("m1", it)])
                nc.scalar.activation(
                    prod1[:], prod1[:], Copy, accum_out=zq[q][:, 1:2]
                ).then_inc(act_sem, 1)
                if it + 3 < NPT:
                    # slot s free once tile it's last xt reader (oc) retired
                    scalar.wait_ge(vec_sem, VC[("oc", it)])
                    rn = bass.ts(tid(it + 3), P)
                    scalar.dma_start(xt[s][:], x_in[rn, :]).then_inc(
                        inx[s], 16
                    )

        @block.sync
        def _(sync):
            gw_flat = gw_in[:, :].rearrange("d t -> (d t)")
            sync.dma_start(
                gwb[:],
                bass.AP(gw_flat.tensor, gw_flat.offset, [[0, P], [1, 2 * D]]),
            ).then_inc(setup_sem, 16)
            gb_flat = gb_in[:]
            sync.dma_start(
                bb[:], bass.AP(gb_flat.tensor, gb_flat.offset, [[0, P], [1, 2]])
            ).then_inc(setup_sem, 16)
            for it in range(NPT):
                s = it % 3
                r = bass.ts(tid(it), P)
                sync.wait_ge(vec_sem, VC[("fence", it)])
                sync.dma_start(out_d[r, :], osl[s][:]).then_inc(sout[s], 16)
            for j in range(3):
                if n_stores(j):
                    sync.wait_ge(sout[j], 16 * n_stores(j))

    nc.finalize()
    return nc


def _get_nc(n_pass=1):
    key = ("nc", n_pass)
    if key not in _CACHE:
        _CACHE[key] = _build(n_pass=n_pass)
    return _CACHE[key]


def _get_runner(n_pass=1):
    """Build (once) a jitted 8-core shard_map runner for the bass module,
    mirroring bass2jax.run_bass_via_pjrt but cached across calls."""
    key = ("fn", n_pass)
    if key in _CACHE:
        return _CACHE[key]
    import jax
    from jax.sharding import Mesh, PartitionSpec
    from jax.experimental.shard_map import shard_map
    from concourse import bass2jax

    nc = _get_nc(n_pass)
    bass2jax.install_neuronx_cc_hook()
    partition_name = (
        nc.partition_id_tensor.name if nc.partition_id_tensor else None
    )
    in_names, out_names, out_avals = [], [], []
    for alloc in nc.m.functions[0].allocations:
        if not isinstance(alloc, mybir.MemoryLocationSet):
            continue
        name = alloc.memorylocations[0].name
        if alloc.kind == "ExternalInput":
            if name != partition_name:
                in_names.append(name)
        elif alloc.kind == "ExternalOutput":
            out_names.append(name)
            shape = tuple(alloc.tensor_shape)
            out_avals.append(
                jax.core.ShapedArray(shape, mybir.dt.np(alloc.dtype))
            )
    n_params = len(in_names)
    n_outs = len(out_avals)
    all_names = in_names + out_names
    if partition_name is not None:
        all_names.append(partition_name)
    donate = tuple(range(n_params, n_params + n_outs))

    def _body(*args):
        operands = list(args)
        if partition_name is not None:
            operands.append(bass2jax.partition_id_tensor())
        outs = bass2jax._bass_exec_p.bind(
            *operands,
            out_avals=tuple(out_avals),
            in_names=tuple(all_names),
            out_names=tuple(out_names),
            lowering_input_output_aliases=(),
            sim_require_finite=True,
            sim_require_nnan=True,
            nc=nc,
        )
        return tuple(outs)

    devices = jax.devices()[:N_CORES]
    mesh = Mesh(np.asarray(devices), ("core",))
    fn = jax.jit(
        shard_map(
            _body,
            mesh=mesh,
            in_specs=(PartitionSpec("core"),) * (n_params + n_outs),
            out_specs=(PartitionSpec("core"),) * n_outs,
            check_rep=False,
        ),
        donate_argnums=donate,
        keep_unused=True,
    )
    runner = (fn, in_names, out_names, out_avals)
    _CACHE[key] = runner
    return runner


def _run_fast(x, gate_w, gate_b, n_pass=1):
    """Execute via the cached jitted runner; returns (x0, x1, combined)."""
    fn, in_names, out_names, out_avals = _get_runner(n_pass)
    full = {"x": x, "gate_w": gate_w, "gate_b": gate_b}
    concat_in = []
    for nm in in_names:
        if nm == "x":
            concat_in.append(x)  # already [N, D]; shard_map splits axis 0
        else:
            a = full[nm]
            concat_in.append(np.concatenate([a] * N_CORES, axis=0))
    zeros = [
        np.zeros((N_CORES * av.shape[0], *av.shape[1:]), av.dtype)
        for av in out_avals
    ]
    outs = fn(*concat_in, *zeros)
    by_name = {nm: np.asarray(o) for nm, o in zip(out_names, outs)}
    arr = by_name["out"].reshape(N, 3, D)
    return (
        arr[:, 0, :].astype(np.float32),
        arr[:, 1, :].astype(np.float32),
        arr[:, 2, :].astype(np.float32),
    )


def _run(x, gate_w, gate_b, trace=False, n_pass=1, **kw):
    x = np.ascontiguousarray(np.asarray(x, dtype=np.float32))
    gate_w = np.ascontiguousarray(np.asarray(gate_w, dtype=np.float32))
    gate_b = np.ascontiguousarray(np.asarray(gate_b, dtype=np.float32))
    assert x.shape == (N, D) and gate_w.shape == (D, 2) and gate_b.shape == (2,)

    nc = _get_nc(n_pass)
    in_maps = [
        {
            "x": x[c * SHARD : (c + 1) * SHARD],
            "gate_w": gate_w,
            "gate_b": gate_b,
        }
        for c in range(N_CORES)
    ]
    res = run_bass_kernel_spmd(
        nc, in_maps, core_ids=list(range(N_CORES)), trace=trace, **kw
    )
    full = np.concatenate(
        [res.results[c]["out"] for c in range(N_CORES)], axis=0
    )
    arr = full.reshape(N, 3, D)
    return (
        arr[:, 0, :].astype(np.float32),
        arr[:, 1, :].astype(np.float32),
        arr[:, 2, :].astype(np.float32),
    ), res


def kernel(x, gate_w, gate_b):
    x = np.ascontiguousarray(np.asarray(x, dtype=np.float32))
    gate_w = np.ascontiguousarray(np.asarray(gate_w, dtype=np.float32))
    gate_b = np.ascontiguousarray(np.asarray(gate_b, dtype=np.float32))
    assert x.shape == (N, D) and gate_w.shape == (D, 2) and gate_b.shape == (2,)
    return _run_fast(x, gate_w, gate_b)


# revision 9
# speedup vs baseline: 1.4991x; 1.4991x over previous
"""BranchRoute (2-branch threshold MoE routing) Trainium2 kernel.

Full-input contract: kernel(x, gate_w, gate_b) -> (x0, x1, combined),
x: [8192, 4096] f32, gate_w: [4096, 2] f32, gate_b: [2] f32.

Math: z = x @ gate_w; m_i = z_i > -b_i  (== sigmoid(z_i + b_i) > 0.5);
x0 = x * m0, x1 = x * m1, combined = x * (m0 + m1).

Sharding: data-parallel over tokens, 8 shards of 1024 tokens, one per
NeuronCore; gate weights replicated; no cross-core communication.

Memory-bound problem, so the kernel minimizes HBM traffic and DMA
instruction count:

  * Outputs are stored as float16 (the correctness budget is generous:
    fp16 round-off is ~3e-4 norm-relative) and widened to f32 on the
    host during the unshard. Write traffic halves: 48 -> 24 MiB/core.
  * The three per-tile outputs live interleaved in one SBUF tensor
    [128, 3*4096] f16 and in one DRAM tensor [SHARD, 3*4096] f16, so
    each tile needs exactly ONE 3 MiB store (columns 0:D = x0,
    D:2D = x1, 2D:3D = combined; host splits via reshape).
  * Per tile: one 2 MiB x load (ACT HWDGE ring, issued by the scalar
    engine) + one 3 MiB store (SP HWDGE ring). 5 MiB/tile total,
    40 MiB/core -> ~106 us at the ~394 GB/s/core the baseline
    f32 kernel measured.

Engine split (per 128-token tile, all under the ~13 us DMA time):
  DVE: two scalar_tensor_tensor ops (prod = x*w into PSUM scratch with
    accum_out -> z, fusing the old mult+ACT-reduce pair), the is_gt
    mask, m0+m1, and the x0 output (tensor_scalar f32 2x mode).
  ACT: the x1 and combined outputs (Copy with per-partition scale,
    1 elem/cycle @ 1.2 GHz).

Raw Bass (no Tile: the local walrus build encodes at most ONE sem wait
per instruction). Per-slot DMA semaphores so every semaphore tracks at
most one outstanding transfer and waits are unambiguous.
"""

import sys

import numpy as np

sys.path.insert(0, "/opt/trn_rl_repo")

import concourse.bass as bass
from concourse import mybir
from concourse.bass_utils import run_bass_kernel_spmd

N_CORES = 8
N, D = 8192, 4096
SHARD = N // N_CORES  # 1024 tokens per core
P = 128
NT = SHARD // P  # 8 tiles per core
F32 = mybir.dt.float32
F16 = mybir.dt.float16
Copy = mybir.ActivationFunctionType.Copy
Alu = mybir.AluOpType

_CACHE = {}


def _build(nt=NT, n_pass=1):
    nc = bass.Bass()
    x_in = nc.dram_tensor("x", [SHARD, D], F32, kind="ExternalInput")
    gw_in = nc.dram_tensor("gate_w", [D, 2], F32, kind="ExternalInput")
    gb_in = nc.dram_tensor("gate_b", [2], F32, kind="ExternalInput")
    out_d = nc.dram_tensor("out", [SHARD, 3 * D], F16, kind="ExternalOutput")

    NPT = nt * n_pass  # total tile iterations (n_pass > 1: timing loops)

    def tid(it):  # tile row index within the shard for iteration it
        return it % nt

    from contextlib import ExitStack

    with ExitStack() as ctx:
        sb = lambda name, *shape, dt=F32: ctx.enter_context(
            nc.sbuf_tensor(name, list(shape), dt)
        )
        sem = lambda name: ctx.enter_context(nc.semaphore(name))
        gwb = sb("gwb", P, 2 * D)  # interleaved w0/w1 bcast
        bb = sb("bb", P, 2)  # bias bcast
        nb = sb("nb", P, 2)  # -bias
        xt = [sb(f"xt{i}", P, D) for i in range(3)]
        osl = [sb(f"osl{i}", P, 3 * D, dt=F16) for i in range(3)]
        scratch = ctx.enter_context(nc.psum_tensor("scratch", [P, D], F32))
        z = sb("z", P, 2)
        mk = [sb(f"mk{j}", P, 3) for j in range(2)]  # m0|m1|ms, dbl-buf
        setup_sem = sem("setup_sem")
        inx = [sem(f"inx{i}") for i in range(3)]
        sout = [sem(f"sout{i}") for i in range(3)]
        vec_sem = sem("vec_sem")
        act_sem = sem("act_sem")
        block = ctx.enter_context(nc.Block())
        # de-interleaved strided views of the broadcast weights [P, D]
        gw_v = gwb[:].rearrange("p (d t) -> p t d", t=2)
        w0v = gw_v[:, 0:1, :].rearrange("p one d -> p (one d)")
        w1v = gw_v[:, 1:2, :].rearrange("p one d -> p (one d)")

        # Engine split: DVE runs the two fused gate ops (prod = x*w into
        # PSUM scratch with accum_out -> z) plus the tiny mask ops; ACT
        # runs ALL THREE masked outputs (Copy with per-partition scale,
        # f32 -> f16) followed by a drain, so a single cheap ACT fence
        # guards the whole osl slot before the store. The DVE emits no
        # drain at all: an InstDrain on a program with accum-bearing DVE
        # ops measured ~4 us/tile extra (accumulator flush), while the
        # ACT drain after plain activations is ~free.
        #
        # The store must not read osl until the writes drained (an op's
        # then_inc can fire while its SBUF writes are still landing --
        # observed as partial-mix corruption of the last-written third),
        # hence the fence; compute-to-compute ordering via semaphores
        # alone is safe (is_gt reads z after stt's inc, ACT reads mk
        # after add's inc).
        #
        #   vec_sem: setup nb op = 1; then 4 ops/tile -> 1+4*it+k, k=1..4
        #     (k: 1 stt0, 2 stt1, 3 is_gt, 4 add)
        #   act_sem: 4 ops/tile -> 4*it+k (k: 1 o1, 2 oc, 3 o0, 4 drain)
        #   inx[s]/sout[s]: per-slot DMA sems as before
        V = lambda it, k: 1 + 4 * it + k
        A = lambda it, k: 4 * it + k

        def x_done(it):  # x-load completions for slot it%3 up to tile it
            return 16 * (it // 3 + 1)

        def slot_free(it):  # store completions needed so slot it%3 is free
            return 16 * (it // 3)

        def n_stores(j):  # stores on slot j over the whole program
            return (NPT - j + 2) // 3

        @block.vector
        def _(vector):
            vector.wait_ge(setup_sem, 32)
            nc.vector.tensor_scalar_mul(nb[:], bb[:], -1.0).then_inc(vec_sem, 1)
            for it in range(NPT):
                s = it % 3
                p = it % 2
                vector.wait_ge(inx[s], x_done(it))
                nc.vector.scalar_tensor_tensor(
                    scratch[:], xt[s][:], 1.0, w0v, Alu.mult, Alu.mult,
                    accum_out=z[:, 0:1],
                ).then_inc(vec_sem, 1)
                nc.vector.scalar_tensor_tensor(
                    scratch[:], xt[s][:], 1.0, w1v, Alu.mult, Alu.mult,
                    accum_out=z[:, 1:2],
                ).then_inc(vec_sem, 1)
                if it >= 2:
                    vector.wait_ge(act_sem, A(it - 2, 3))  # mk[p] consumed
                vector.wait_ge(vec_sem, V(it, 2))  # z writes drained
                nc.vector.tensor_tensor(
                    mk[p][:, 0:2], z[:, 0:2], nb[:, 0:2], Alu.is_gt
                ).then_inc(vec_sem, 1)
                vector.wait_ge(vec_sem, V(it, 3))  # m writes drained
                nc.vector.tensor_add(
                    mk[p][:, 2:3], mk[p][:, 0:1], mk[p][:, 1:2]
                ).then_inc(vec_sem, 1)

        @block.scalar
        def _(scalar):
            # x loads ride the Activation HWDGE ring so they never queue
            # behind store waits on the SP ring.
            for it in range(min(3, NPT)):
                r = bass.ts(tid(it), P)
                scalar.dma_start(xt[it][:], x_in[r, :]).then_inc(inx[it], 16)
            for it in range(NPT):
                s = it % 3
                p = it % 2
                scalar.wait_ge(vec_sem, V(it, 4))  # m0/m1/ms ready
                if it >= 3:
                    scalar.wait_ge(sout[s], slot_free(it))  # osl[s] stored
                nc.scalar.activation(
                    osl[s][:, D : 2 * D], xt[s][:], Copy, scale=mk[p][:, 1:2]
                ).then_inc(act_sem, 1)
                nc.scalar.activation(
                    osl[s][:, 2 * D : 3 * D], xt[s][:], Copy, scale=mk[p][:, 2:3]
                ).then_inc(act_sem, 1)
                nc.scalar.activation(
                    osl[s][:, 0:D], xt[s][:], Copy, scale=mk[p][:, 0:1]
                ).then_inc(act_sem, 1)
                nc.scalar.drain().then_inc(act_sem, 1)  # osl writes fenced
                if it + 3 < NPT:
                    # xt[s] readers all retired: DVE stts done (implied by
                    # the V(it,4) mask wait), ACT o* retired in order
                    rn = bass.ts(tid(it + 3), P)
                    scalar.dma_start(xt[s][:], x_in[rn, :]).then_inc(
                        inx[s], 16
                    )

        @block.sync
        def _(sync):
            gw_flat = gw_in[:, :].rearrange("d t -> (d t)")
            sync.dma_start(
                gwb[:],
                bass.AP(gw_flat.tensor, gw_flat.offset, [[0, P], [1, 2 * D]]),
            ).then_inc(setup_sem, 16)
            gb_flat = gb_in[:]
            sync.dma_start(
                bb[:], bass.AP(gb_flat.tensor, gb_flat.offset, [[0, P], [1, 2]])
            ).then_inc(setup_sem, 16)
            for it in range(NPT):
                s = it % 3
                r = bass.ts(tid(it), P)
                sync.wait_ge(act_sem, A(it, 4))
                sync.dma_start(out_d[r, :], osl[s][:]).then_inc(sout[s], 16)
            for j in range(3):
                if n_stores(j):
                    sync.wait_ge(sout[j], 16 * n_stores(j))

    nc.finalize()
    return nc


def _get_nc(n_pass=1):
    key = ("nc", n_pass)
    if key not in _CACHE:
        _CACHE[key] = _build(n_pass=n_pass)
    return _CACHE[key]


def _get_runner(n_pass=1):
    """Build (once) a jitted 8-core shard_map runner for the bass module,
    mirroring bass2jax.run_bass_via_pjrt but cached across calls."""
    key = ("fn", n_pass)
    if key in _CACHE:
        return _CACHE[key]
    import jax
    from jax.sharding import Mesh, PartitionSpec
    from jax.experimental.shard_map import shard_map
    from concourse import bass2jax

    nc = _get_nc(n_pass)
    bass2jax.install_neuronx_cc_hook()
    partition_name = (
        nc.partition_id_tensor.name if nc.partition_id_tensor else None
    )
    in_names, out_names, out_avals = [], [], []
    for alloc in nc.m.functions[0].allocations:
        if not isinstance(alloc, mybir.MemoryLocationSet):
            continue
        name = alloc.memorylocations[0].name
        if alloc.kind == "ExternalInput":
            if name != partition_name:
                in_names.append(name)
        elif alloc.kind == "ExternalOutput":
            out_names.append(name)
            shape = tuple(alloc.tensor_shape)
            out_avals.append(
                jax.core.ShapedArray(shape, mybir.dt.np(alloc.dtype))
            )
    n_params = len(in_names)
    n_outs = len(out_avals)
    all_names = in_names + out_names
    if partition_name is not None:
        all_names.append(partition_name)
    donate = tuple(range(n_params, n_params + n_outs))

    def _body(*args):
        operands = list(args)
        if partition_name is not None:
            operands.append(bass2jax.partition_id_tensor())
        outs = bass2jax._bass_exec_p.bind(
            *operands,
            out_avals=tuple(out_avals),
            in_names=tuple(all_names),
            out_names=tuple(out_names),
            lowering_input_output_aliases=(),
            sim_require_finite=True,
            sim_require_nnan=True,
            nc=nc,
        )
        return tuple(outs)

    devices = jax.devices()[:N_CORES]
    mesh = Mesh(np.asarray(devices), ("core",))
    fn = jax.jit(
        shard_map(
            _body,
            mesh=mesh,
            in_specs=(PartitionSpec("core"),) * (n_params + n_outs),
            out_specs=(PartitionSpec("core"),) * n_outs,
            check_rep=False,
        ),
        donate_argnums=donate,
        keep_unused=True,
    )
    runner = (fn, in_names, out_names, out_avals)
    _CACHE[key] = runner
    return runner


def _run_fast(x, gate_w, gate_b, n_pass=1):
    """Execute via the cached jitted runner; returns (x0, x1, combined)."""
    fn, in_names, out_names, out_avals = _get_runner(n_pass)
    full = {"x": x, "gate_w": gate_w, "gate_b": gate_b}
    concat_in = []
    for nm in in_names:
        if nm == "x":
            concat_in.append(x)  # already [N, D]; shard_map splits axis 0
        else:
            a = full[nm]
            concat_in.append(np.concatenate([a] * N_CORES, axis=0))
    zeros = [
        np.zeros((N_CORES * av.shape[0], *av.shape[1:]), av.dtype)
        for av in out_avals
    ]
    outs = fn(*concat_in, *zeros)
    by_name = {nm: np.asarray(o) for nm, o in zip(out_names, outs)}
    arr = by_name["out"].reshape(N, 3, D)
    return (
        arr[:, 0, :].astype(np.float32),
        arr[:, 1, :].astype(np.float32),
        arr[:, 2, :].astype(np.float32),
    )


def _run(x, gate_w, gate_b, trace=False, n_pass=1, **kw):
    x = np.ascontiguousarray(np.asarray(x, dtype=np.float32))
    gate_w = np.ascontiguousarray(np.asarray(gate_w, dtype=np.float32))
    gate_b = np.ascontiguousarray(np.asarray(gate_b, dtype=np.float32))
    assert x.shape == (N, D) and gate_w.shape == (D, 2) and gate_b.shape == (2,)

    nc = _get_nc(n_pass)
    in_maps = [
        {
            "x": x[c * SHARD : (c + 1) * SHARD],
            "gate_w": gate_w,
            "gate_b": gate_b,
        }
        for c in range(N_CORES)
    ]
    res = run_bass_kernel_spmd(
        nc, in_maps, core_ids=list(range(N_CORES)), trace=trace, **kw
    )
    full = np.concatenate(
        [res.results[c]["out"] for c in range(N_CORES)], axis=0
    )
    arr = full.reshape(N, 3, D)
    return (
        arr[:, 0, :].astype(np.float32),
        arr[:, 1, :].astype(np.float32),
        arr[:, 2, :].astype(np.float32),
    ), res


def kernel(x, gate_w, gate_b):
    x = np.ascontiguousarray(np.asarray(x, dtype=np.float32))
    gate_w = np.ascontiguousarray(np.asarray(gate_w, dtype=np.float32))
    gate_b = np.ascontiguousarray(np.asarray(gate_b, dtype=np.float32))
    assert x.shape == (N, D) and gate_w.shape == (D, 2) and gate_b.shape == (2,)
    return _run_fast(x, gate_w, gate_b)
